# revision 14
# baseline (speedup 1.0000x reference)
"""Bass/Trainium2 kernel for nn_Attention_75007308857927.

Reference computation (B=4, S=2048, D=1024):
    Q = X @ Wq.T ; K = X @ Wk.T ; V = X @ Wv.T         (per batch)
    Qn, Kn = row-normalized Q, K
    scores = (Qn @ Kn.T) * m      m[i,j] = 1 if (j > i) or masks[j]==0 else 0
    out = scores @ V

Sharding: 8 cores = 4 batches x 2 query/key-halves. Each core projects
K/V/Q from its own 1024-row half of X; the full KT and V' (V scaled by
1/||K||) are assembled across the core pair with an AllGather, then each
core computes scores/out for its query half.

Device algebra per core (matmuls contract over the partition dim):
    KT[e,j']  = sum_d WkT[d,e] * XQ[d,j']        (own keys j', KT e-major)
    kinv[j']  = rsqrt(sum_e KT[e,j']^2)
    V'[j',e]  = (sum_d XQ[d,j'] WvT[d,e]) * kinv[j']
    KT, V'    = AllGather over the core pair     (global key order)
    QT[e,i]   = sum_d WqT[d,e] * XQ[d,i]
    qinv[i]   = rsqrt(sum_e QT[e,i]^2)
    ST[j,i]   = (sum_e KT[e,j] QT[e,i]) * maskT[j,i]   (mask fused in evict)
    out[i,d]  = (sum_j ST[j,i] V'[j,d]) * qinv[i]

bf16 matmul operands, f32 PSUM accumulation. Precision vs f32 reference:
absmax error ~0.4% of output scale.
"""

import numpy as np
import ml_dtypes

B, S, D = 4, 2048, 1024
HALF = S // 2  # queries/keys per core
N_CORES = 8
P = 128
DC = D // P    # 8 contraction chunks over d
ET = D // P    # 8 e-tiles
JT = S // P    # 16 j-tiles (global)
JTH = HALF // P  # 8 own j-tiles
I5 = HALF // 512  # 2

BF16 = ml_dtypes.bfloat16

_CACHE = {}


def _emit(ctx, tc, xq, wkt, wvt, wqt, maskt, out, kt_own, kt_gath, v_own, v_gath):
    from concourse import mybir

    nc = tc.nc
    dtb = mybir.dt.bfloat16
    dtf = mybir.dt.float32

    # ---- SBUF pools -------------------------------------------------------
    xq_p = ctx.enter_context(tc.tile_pool(name="xq", bufs=1))
    # weights + scores blocks share one pool: all tiles are 16KB/partition
    w_p = ctx.enter_context(tc.tile_pool(name="wst", bufs=3))
    kt_p = ctx.enter_context(tc.tile_pool(name="kt", bufs=1))
    qt_p = ctx.enter_context(tc.tile_pool(name="qt", bufs=1))
    vp_p = ctx.enter_context(tc.tile_pool(name="vp", bufs=1))
    row_p = ctx.enter_context(tc.tile_pool(name="rows", bufs=1))
    sq_p = ctx.enter_context(tc.tile_pool(name="sq", bufs=3))
    stg_p = ctx.enter_context(tc.tile_pool(name="stg", bufs=6))
    mk_p = ctx.enter_context(tc.tile_pool(name="mk", bufs=6))
    ev_p = ctx.enter_context(tc.tile_pool(name="ev", bufs=3))
    ps_p = ctx.enter_context(tc.tile_pool(name="psmm", bufs=4, space="PSUM"))
    psr_p = ctx.enter_context(tc.tile_pool(name="psrow", bufs=2, space="PSUM"))
    psc_p = ctx.enter_context(tc.tile_pool(name="pscol", bufs=2, space="PSUM"))

    xq_s = xq_p.tile([P, DC * HALF], dtb, tag="xq")    # [d%128, dc*1024+i]
    wkt_s = w_p.tile([P, DC * D], dtb, tag="w")        # [d%128, dc*1024+e]
    wvt_s = w_p.tile([P, DC * D], dtb, tag="w")
    wqt_s = w_p.tile([P, DC * D], dtb, tag="w")
    kt_s = kt_p.tile([P, ET * S], dtb, tag="kt")       # [e%128, et*2048+j]
    qt_s = qt_p.tile([P, ET * HALF], dtb, tag="qt")    # [e%128, et*1024+i]
    vp_s = vp_p.tile([P, JT * D], dtb, tag="vp")       # [j%128, jt*1024+d]

    ones_b = row_p.tile([P, 1], dtb, tag="ones_b")
    ones_f = row_p.tile([1, 1], dtf, tag="ones_f")
    ksq_row = row_p.tile([1, HALF], dtf, tag="sqrow")
    qsq_row = row_p.tile([1, HALF], dtf, tag="sqrow")
    ksq_col = row_p.tile([P, JTH], dtf, tag="ksqc")    # col c <-> own j-chunk c
    krec_col = row_p.tile([P, JTH], dtf, tag="krecc")
    kinv_col = row_p.tile([P, JTH], dtf, tag="kinvc")
    qsq_col = row_p.tile([P, ET], dtf, tag="qsqc")
    qrec_col = row_p.tile([P, ET], dtf, tag="qrecc")
    qinv_col = row_p.tile([P, ET], dtf, tag="qinvc")

    nc.vector.memset(ones_b[:], 1.0)
    nc.vector.memset(ones_f[:], 1.0)

    # ---- input DMAs (ordered for earliest matmul start: B needs wkt+xq) --
    for dc in range(DC):
        nc.sync.dma_start(wkt_s[:, dc * D:(dc + 1) * D], wkt[dc * P:(dc + 1) * P, :])
        nc.sync.dma_start(xq_s[:, dc * HALF:(dc + 1) * HALF],
                          xq[dc * P:(dc + 1) * P, :])
    for w_s, w_d in ((wvt_s, wvt), (wqt_s, wqt)):
        for dc in range(DC):
            nc.sync.dma_start(w_s[:, dc * D:(dc + 1) * D], w_d[dc * P:(dc + 1) * P, :])

    groups = [[0, 1], [2, 3], [4, 5], [6, 7]]

    # ---- phase B: KT for own keys + k sumsq ------------------------------
    for j5 in range(I5):
        ksq_ps = psr_p.tile([1, 512], dtf, tag="psrow")
        for et in range(ET):
            ps = ps_p.tile([P, 512], dtf, tag="psmm")
            for dc in range(DC):
                nc.tensor.matmul(
                    ps[:],
                    lhsT=wkt_s[:, dc * D + et * P: dc * D + (et + 1) * P],
                    rhs=xq_s[:, dc * HALF + j5 * 512: dc * HALF + j5 * 512 + 512],
                    start=(dc == 0), stop=(dc == DC - 1),
                )
            stg = stg_p.tile([P, 512], dtb, tag="stg")
            nc.vector.tensor_copy(stg[:], ps[:])
            nc.sync.dma_start(kt_own[et * P:(et + 1) * P, j5 * 512: j5 * 512 + 512],
                              stg[:])
            sq = sq_p.tile([P, 512], dtb, tag="sq")
            nc.scalar.square(sq[:], stg[:])
            nc.tensor.matmul(ksq_ps[:], lhsT=ones_b[:], rhs=sq[:],
                             start=(et == 0), stop=(et == ET - 1))
        nc.vector.tensor_copy(ksq_row[0:1, j5 * 512: j5 * 512 + 512], ksq_ps[:])
        for cc in range(4):
            c = j5 * 4 + cc
            pc = psc_p.tile([P, 1], dtf, tag="pscol")
            nc.tensor.matmul(pc[:], lhsT=ksq_row[0:1, c * P:(c + 1) * P],
                             rhs=ones_f[:], start=True, stop=True)
            nc.vector.tensor_copy(ksq_col[:, c:c + 1], pc[:])

    # kinv for own keys, 128-way parallel in column layout
    nc.vector.reciprocal(krec_col[:], ksq_col[:])
    nc.scalar.sqrt(kinv_col[:], krec_col[:])

    # gather KT across the core pair, then load full KT to SBUF
    nc.gpsimd.collective_compute(
        "AllGather", mybir.AluOpType.bypass, replica_groups=groups,
        ins=[kt_own[:]], outs=[kt_gath[:]])
    for r in range(2):
        # one strided DMA per rank: [et, p, j] -> cols et*S + r*HALF + j
        dst = kt_s[:].rearrange("p (et j) -> p et j", et=ET, j=S)[
            :, :, r * HALF:(r + 1) * HALF]
        src_ap = kt_gath[r].rearrange("(et p) j -> p et j", p=P)
        nc.gpsimd.dma_start(dst, src_ap)

    # ---- phase D: V' = V * kinv[j] for own keys --------------------------
    for jt in range(JTH):
        for e5 in range(2):
            ps = ps_p.tile([P, 512], dtf, tag="psmm")
            for dc in range(DC):
                nc.tensor.matmul(
                    ps[:],
                    lhsT=xq_s[:, dc * HALF + jt * P: dc * HALF + (jt + 1) * P],
                    rhs=wvt_s[:, dc * D + e5 * 512: dc * D + e5 * 512 + 512],
                    start=(dc == 0), stop=(dc == DC - 1),
                )
            stg = stg_p.tile([P, 512], dtb, tag="stg")
            nc.vector.tensor_scalar_mul(stg[:], ps[:], kinv_col[:, jt:jt + 1])
            nc.sync.dma_start(v_own[jt * P:(jt + 1) * P, e5 * 512: e5 * 512 + 512],
                              stg[:])

    nc.gpsimd.collective_compute(
        "AllGather", mybir.AluOpType.bypass, replica_groups=groups,
        ins=[v_own[:]], outs=[v_gath[:]])
    for r in range(2):
        dst = vp_s[:, r * JTH * D: (r * JTH + JTH) * D]
        dst = dst.rearrange("p (jtl e) -> p jtl e", jtl=JTH, e=D)
        src_ap = v_gath[r].rearrange("(jtl p) e -> p jtl e", p=P)
        nc.gpsimd.dma_start(dst, src_ap)

    # ---- phase E: QT + q sumsq -------------------------------------------
    for i5 in range(I5):
        qsq_ps = psr_p.tile([1, 512], dtf, tag="psrow")
        for et in range(ET):
            ps = ps_p.tile([P, 512], dtf, tag="psmm")
            for dc in range(DC):
                nc.tensor.matmul(
                    ps[:],
                    lhsT=wqt_s[:, dc * D + et * P: dc * D + (et + 1) * P],
                    rhs=xq_s[:, dc * HALF + i5 * 512: dc * HALF + i5 * 512 + 512],
                    start=(dc == 0), stop=(dc == DC - 1),
                )
            qtsl = qt_s[:, et * HALF + i5 * 512: et * HALF + i5 * 512 + 512]
            nc.vector.tensor_copy(qtsl, ps[:])
            sq = sq_p.tile([P, 512], dtb, tag="sq")
            nc.scalar.square(sq[:], qtsl)
            nc.tensor.matmul(qsq_ps[:], lhsT=ones_b[:], rhs=sq[:],
                             start=(et == 0), stop=(et == ET - 1))
        nc.vector.tensor_copy(qsq_row[0:1, i5 * 512: i5 * 512 + 512], qsq_ps[:])
        for cc in range(4):
            c = i5 * 4 + cc
            pc = psc_p.tile([P, 1], dtf, tag="pscol")
            nc.tensor.matmul(pc[:], lhsT=qsq_row[0:1, c * P:(c + 1) * P],
                             rhs=ones_f[:], start=True, stop=True)
            nc.vector.tensor_copy(qsq_col[:, c:c + 1], pc[:])

    nc.vector.reciprocal(qrec_col[:], qsq_col[:])
    nc.scalar.sqrt(qinv_col[:], qrec_col[:])

    # ---- phase F: all score blocks first (hides the V' gather), then ------
    # ---- phase G: all out blocks ------------------------------------------
    st_blks = []
    for ib in range(I5):
        st_blk = w_p.tile([P, JT * 512], dtb, tag="w")  # [j%128, jt*512+i]
        st_blks.append(st_blk)
        for jt in range(JT):
            ps = ps_p.tile([P, 512], dtf, tag="psmm")
            for et in range(ET):
                nc.tensor.matmul(
                    ps[:],
                    lhsT=kt_s[:, et * S + jt * P: et * S + (jt + 1) * P],
                    rhs=qt_s[:, et * HALF + ib * 512: et * HALF + ib * 512 + 512],
                    start=(et == 0), stop=(et == ET - 1),
                )
            mk = mk_p.tile([P, 512], dtb, tag="mk")
            nc.sync.dma_start(mk[:], maskt[jt * P:(jt + 1) * P,
                                           ib * 512: ib * 512 + 512])
            nc.vector.tensor_mul(st_blk[:, jt * 512:(jt + 1) * 512], ps[:], mk[:])
    for ib in range(I5):
        st_blk = st_blks[ib]
        for itl in range(4):
            g = ib * 4 + itl  # global i-tile
            for d5 in range(2):
                ps = ps_p.tile([P, 512], dtf, tag="psmm")
                for jt in range(JT):
                    nc.tensor.matmul(
                        ps[:],
                        lhsT=st_blk[:, jt * 512 + itl * P: jt * 512 + (itl + 1) * P],
                        rhs=vp_s[:, jt * D + d5 * 512: jt * D + d5 * 512 + 512],
                        start=(jt == 0), stop=(jt == JT - 1),
                    )
                ot = ev_p.tile([P, 512], dtf, tag="ev")
                nc.vector.tensor_scalar_mul(ot[:], ps[:], qinv_col[:, g:g + 1])
                nc.sync.dma_start(out[g * P:(g + 1) * P, d5 * 512: d5 * 512 + 512],
                                  ot[:])


def _build():
    if "nc" in _CACHE:
        return _CACHE["nc"]
    import concourse.tile as tile
    from concourse import bacc, mybir

    dtb = mybir.dt.bfloat16
    dtf = mybir.dt.float32
    nc = bacc.Bacc("TRN2", target_bir_lowering=False, debug=False,
                   enable_asserts=True, num_devices=N_CORES)
    xq = nc.dram_tensor("xq", [D, HALF], dtb, kind="ExternalInput").ap()
    wkt = nc.dram_tensor("wkt", [D, D], dtb, kind="ExternalInput").ap()
    wvt = nc.dram_tensor("wvt", [D, D], dtb, kind="ExternalInput").ap()
    wqt = nc.dram_tensor("wqt", [D, D], dtb, kind="ExternalInput").ap()
    maskt = nc.dram_tensor("maskt", [S, HALF], dtb, kind="ExternalInput").ap()
    out = nc.dram_tensor("out", [HALF, D], dtf, kind="ExternalOutput").ap()
    kt_own = nc.dram_tensor("kt_own", [D, HALF], dtb).ap()
    kt_gath = nc.dram_tensor("kt_gath", [2, D, HALF], dtb).ap()
    v_own = nc.dram_tensor("v_own", [HALF, D], dtb).ap()
    v_gath = nc.dram_tensor("v_gath", [2, HALF, D], dtb).ap()

    from contextlib import ExitStack
    with tile.TileContext(nc) as tc:
        with ExitStack() as ctx:
            _emit(ctx, tc, xq, wkt, wvt, wqt, maskt, out,
                  kt_own, kt_gath, v_own, v_gath)
    nc.compile()
    _CACHE["nc"] = nc
    return nc


def make_in_maps(X, masks, Wq, Wk, Wv):
    """Host-side sharding/layout: one input map per core (global key order)."""
    in_maps = []
    wkt_h = np.ascontiguousarray(Wk.T).astype(BF16)
    wvt_h = np.ascontiguousarray(Wv.T).astype(BF16)
    wqt_h = np.ascontiguousarray(Wq.T).astype(BF16)
    for c in range(N_CORES):
        b, h = c // 2, c % 2
        XT = X[b].T.astype(BF16)                                # [D, S]
        j = np.arange(S)[:, None]
        i = h * HALF + np.arange(HALF)[None, :]
        mT = ((j > i) | (masks[b] == 0)[:, None]).astype(BF16)  # [S, HALF]
        in_maps.append({
            "xq": np.ascontiguousarray(XT[:, h * HALF:(h + 1) * HALF]),
            "wkt": wkt_h,
            "wvt": wvt_h,
            "wqt": wqt_h,
            "maskt": mT,
        })
    return in_maps


def run(in_maps, **kw):
    from concourse.bass_utils import run_bass_kernel_spmd
    nc = _build()
    return run_bass_kernel_spmd(nc, in_maps, list(range(N_CORES)), **kw)


def kernel(X, masks, Wq, Wk, Wv):
    X = np.asarray(X, dtype=np.float32)
    masks = np.asarray(masks)
    res = run(make_in_maps(X, masks, np.asarray(Wq, np.float32),
                           np.asarray(Wk, np.float32), np.asarray(Wv, np.float32)))
    out = np.empty((B, S, D), np.float32)
    for c in range(N_CORES):
        b, h = c // 2, c % 2
        out[b, h * HALF:(h + 1) * HALF, :] = res.results[c]["out"]
    return out


# revision 15
# speedup vs baseline: 1.0570x; 1.0570x over previous
"""Bass/Trainium2 kernel for nn_Attention_75007308857927.

Reference computation (B=4, S=2048, D=1024):
    Q = X @ Wq.T ; K = X @ Wk.T ; V = X @ Wv.T         (per batch)
    Qn, Kn = row-normalized Q, K
    scores = (Qn @ Kn.T) * m      m[i,j] = 1 if (j > i) or masks[j]==0 else 0
    out = scores @ V

Sharding: 8 cores = 4 batches x 2 query/key-halves. Each core projects
K/V/Q from its own 1024-row half of X; the full KT and V' (V scaled by
1/||K||) are assembled across the core pair with an AllGather, then each
core computes scores/out for its query half.

Device algebra per core (matmuls contract over the partition dim):
    KT[e,j']  = sum_d WkT[d,e] * XQ[d,j']        (own keys j', KT e-major)
    kinv[j']  = rsqrt(sum_e KT[e,j']^2)
    V'[j',e]  = (sum_d XQ[d,j'] WvT[d,e]) * kinv[j']
    KT, V'    = AllGather over the core pair     (global key order)
    QT[e,i]   = sum_d WqT[d,e] * XQ[d,i]
    qinv[i]   = rsqrt(sum_e QT[e,i]^2)
    ST[j,i]   = (sum_e KT[e,j] QT[e,i]) * maskT[j,i]   (mask fused in evict)
    out[i,d]  = (sum_j ST[j,i] V'[j,d]) * qinv[i]

bf16 matmul operands, f32 PSUM accumulation. Precision vs f32 reference:
absmax error ~0.4% of output scale.
"""

import numpy as np
import ml_dtypes

B, S, D = 4, 2048, 1024
HALF = S // 2  # queries/keys per core
N_CORES = 8
P = 128
DC = D // P    # 8 contraction chunks over d
ET = D // P    # 8 e-tiles
JT = S // P    # 16 j-tiles (global)
JTH = HALF // P  # 8 own j-tiles
I5 = HALF // 512  # 2

BF16 = ml_dtypes.bfloat16

_CACHE = {}


def _emit(ctx, tc, xq, wkt, wvt, wqt, maskt, out, kt_own, kt_gath, v_own, v_gath):
    from concourse import mybir

    nc = tc.nc
    dtb = mybir.dt.bfloat16
    dtf = mybir.dt.float32

    # ---- SBUF pools -------------------------------------------------------
    xq_p = ctx.enter_context(tc.tile_pool(name="xq", bufs=1))
    # weights + scores blocks share one pool: all tiles are 16KB/partition
    w_p = ctx.enter_context(tc.tile_pool(name="wst", bufs=3))
    kt_p = ctx.enter_context(tc.tile_pool(name="kt", bufs=1))
    qt_p = ctx.enter_context(tc.tile_pool(name="qt", bufs=1))
    vp_p = ctx.enter_context(tc.tile_pool(name="vp", bufs=1))
    row_p = ctx.enter_context(tc.tile_pool(name="rows", bufs=1))
    sq_p = ctx.enter_context(tc.tile_pool(name="sq", bufs=3))
    stg_p = ctx.enter_context(tc.tile_pool(name="stg", bufs=6))
    mk_p = ctx.enter_context(tc.tile_pool(name="mk", bufs=6))
    ev_p = ctx.enter_context(tc.tile_pool(name="ev", bufs=3))
    ps_p = ctx.enter_context(tc.tile_pool(name="psmm", bufs=4, space="PSUM"))
    psr_p = ctx.enter_context(tc.tile_pool(name="psrow", bufs=2, space="PSUM"))
    psc_p = ctx.enter_context(tc.tile_pool(name="pscol", bufs=2, space="PSUM"))

    xq_s = xq_p.tile([P, DC * HALF], dtb, tag="xq")    # [d%128, dc*1024+i]
    wkt_s = w_p.tile([P, DC * D], dtb, tag="w")        # [d%128, dc*1024+e]
    wvt_s = w_p.tile([P, DC * D], dtb, tag="w")
    wqt_s = w_p.tile([P, DC * D], dtb, tag="w")
    kt_s = kt_p.tile([P, ET * S], dtb, tag="kt")       # [e%128, et*2048+j]
    qt_s = qt_p.tile([P, ET * HALF], dtb, tag="qt")    # [e%128, et*1024+i]
    vp_s = vp_p.tile([P, JT * D], dtb, tag="vp")       # [j%128, jt*1024+d]

    ones_b = row_p.tile([P, 1], dtb, tag="ones_b")
    ones_f = row_p.tile([1, 1], dtf, tag="ones_f")
    ksq_row = row_p.tile([1, HALF], dtf, tag="sqrow")
    qsq_row = row_p.tile([1, HALF], dtf, tag="sqrow")
    ksq_col = row_p.tile([P, JTH], dtf, tag="ksqc")    # col c <-> own j-chunk c
    krec_col = row_p.tile([P, JTH], dtf, tag="krecc")
    kinv_col = row_p.tile([P, JTH], dtf, tag="kinvc")
    qsq_col = row_p.tile([P, ET], dtf, tag="qsqc")
    qrec_col = row_p.tile([P, ET], dtf, tag="qrecc")
    qinv_col = row_p.tile([P, ET], dtf, tag="qinvc")

    nc.vector.memset(ones_b[:], 1.0)
    nc.vector.memset(ones_f[:], 1.0)

    # ---- input DMAs (ordered for earliest matmul start: B needs wkt+xq) --
    for dc in range(DC):
        nc.sync.dma_start(wkt_s[:, dc * D:(dc + 1) * D], wkt[dc * P:(dc + 1) * P, :])
        nc.sync.dma_start(xq_s[:, dc * HALF:(dc + 1) * HALF],
                          xq[dc * P:(dc + 1) * P, :])
    for w_s, w_d in ((wvt_s, wvt), (wqt_s, wqt)):
        for dc in range(DC):
            nc.sync.dma_start(w_s[:, dc * D:(dc + 1) * D], w_d[dc * P:(dc + 1) * P, :])

    groups = [[0, 1], [2, 3], [4, 5], [6, 7]]

    # ---- phase B: KT for own keys + k sumsq ------------------------------
    for j5 in range(I5):
        ksq_ps = psr_p.tile([1, 512], dtf, tag="psrow")
        for et in range(ET):
            ps = ps_p.tile([P, 512], dtf, tag="psmm")
            for dc in range(DC):
                nc.tensor.matmul(
                    ps[:],
                    lhsT=wkt_s[:, dc * D + et * P: dc * D + (et + 1) * P],
                    rhs=xq_s[:, dc * HALF + j5 * 512: dc * HALF + j5 * 512 + 512],
                    start=(dc == 0), stop=(dc == DC - 1),
                )
            stg = stg_p.tile([P, 512], dtb, tag="stg")
            nc.vector.tensor_copy(stg[:], ps[:])
            nc.sync.dma_start(kt_own[et * P:(et + 1) * P, j5 * 512: j5 * 512 + 512],
                              stg[:])
            sq = sq_p.tile([P, 512], dtb, tag="sq")
            nc.scalar.square(sq[:], stg[:])
            nc.tensor.matmul(ksq_ps[:], lhsT=ones_b[:], rhs=sq[:],
                             start=(et == 0), stop=(et == ET - 1))
        nc.vector.tensor_copy(ksq_row[0:1, j5 * 512: j5 * 512 + 512], ksq_ps[:])
        for cc in range(4):
            c = j5 * 4 + cc
            pc = psc_p.tile([P, 1], dtf, tag="pscol")
            nc.tensor.matmul(pc[:], lhsT=ksq_row[0:1, c * P:(c + 1) * P],
                             rhs=ones_f[:], start=True, stop=True)
            nc.vector.tensor_copy(ksq_col[:, c:c + 1], pc[:])

    # kinv for own keys, 128-way parallel in column layout
    nc.vector.reciprocal(krec_col[:], ksq_col[:])
    nc.scalar.sqrt(kinv_col[:], krec_col[:])

    # gather KT across the core pair, then load full KT to SBUF
    nc.gpsimd.collective_compute(
        "AllGather", mybir.AluOpType.bypass, replica_groups=groups,
        ins=[kt_own[:]], outs=[kt_gath[:]])
    kt3 = kt_s[:].rearrange("p (et j) -> p et j", et=ET, j=S)
    for r in range(2):
        src3 = kt_gath[r].rearrange("(et p) j -> p et j", p=P)
        for eg in range(0, ET, 2):
            nc.gpsimd.dma_start(
                kt3[:, eg:eg + 2, r * HALF:(r + 1) * HALF],
                src3[:, eg:eg + 2, :])

    # ---- phase D: V' = V * kinv[j] for own keys --------------------------
    for jt in range(JTH):
        for e5 in range(2):
            ps = ps_p.tile([P, 512], dtf, tag="psmm")
            for dc in range(DC):
                nc.tensor.matmul(
                    ps[:],
                    lhsT=xq_s[:, dc * HALF + jt * P: dc * HALF + (jt + 1) * P],
                    rhs=wvt_s[:, dc * D + e5 * 512: dc * D + e5 * 512 + 512],
                    start=(dc == 0), stop=(dc == DC - 1),
                )
            stg = stg_p.tile([P, 512], dtb, tag="stg")
            nc.vector.tensor_scalar_mul(stg[:], ps[:], kinv_col[:, jt:jt + 1])
            nc.sync.dma_start(v_own[jt * P:(jt + 1) * P, e5 * 512: e5 * 512 + 512],
                              stg[:])

    nc.gpsimd.collective_compute(
        "AllGather", mybir.AluOpType.bypass, replica_groups=groups,
        ins=[v_own[:]], outs=[v_gath[:]])
    for r in range(2):
        dst = vp_s[:, r * JTH * D: (r * JTH + JTH) * D]
        dst = dst.rearrange("p (jtl e) -> p jtl e", jtl=JTH, e=D)
        src_ap = v_gath[r].rearrange("(jtl p) e -> p jtl e", p=P)
        for jg in range(0, JTH, 2):
            nc.gpsimd.dma_start(dst[:, jg:jg + 2, :], src_ap[:, jg:jg + 2, :])

    # ---- phase E: QT + q sumsq -------------------------------------------
    for i5 in range(I5):
        qsq_ps = psr_p.tile([1, 512], dtf, tag="psrow")
        for et in range(ET):
            ps = ps_p.tile([P, 512], dtf, tag="psmm")
            for dc in range(DC):
                nc.tensor.matmul(
                    ps[:],
                    lhsT=wqt_s[:, dc * D + et * P: dc * D + (et + 1) * P],
                    rhs=xq_s[:, dc * HALF + i5 * 512: dc * HALF + i5 * 512 + 512],
                    start=(dc == 0), stop=(dc == DC - 1),
                )
            qtsl = qt_s[:, et * HALF + i5 * 512: et * HALF + i5 * 512 + 512]
            nc.vector.tensor_copy(qtsl, ps[:])
            sq = sq_p.tile([P, 512], dtb, tag="sq")
            nc.scalar.square(sq[:], qtsl)
            nc.tensor.matmul(qsq_ps[:], lhsT=ones_b[:], rhs=sq[:],
                             start=(et == 0), stop=(et == ET - 1))
        nc.vector.tensor_copy(qsq_row[0:1, i5 * 512: i5 * 512 + 512], qsq_ps[:])
        for cc in range(4):
            c = i5 * 4 + cc
            pc = psc_p.tile([P, 1], dtf, tag="pscol")
            nc.tensor.matmul(pc[:], lhsT=qsq_row[0:1, c * P:(c + 1) * P],
                             rhs=ones_f[:], start=True, stop=True)
            nc.vector.tensor_copy(qsq_col[:, c:c + 1], pc[:])

    nc.vector.reciprocal(qrec_col[:], qsq_col[:])
    nc.scalar.sqrt(qinv_col[:], qrec_col[:])

    # ---- phase F: all score blocks first (hides the V' gather), then ------
    # ---- phase G: all out blocks ------------------------------------------
    st_blks = []
    for ib in range(I5):
        st_blk = w_p.tile([P, JT * 512], dtb, tag="w")  # [j%128, jt*512+i]
        st_blks.append(st_blk)
        for jt in range(JT):
            ps = ps_p.tile([P, 512], dtf, tag="psmm")
            for et in range(ET):
                nc.tensor.matmul(
                    ps[:],
                    lhsT=kt_s[:, et * S + jt * P: et * S + (jt + 1) * P],
                    rhs=qt_s[:, et * HALF + ib * 512: et * HALF + ib * 512 + 512],
                    start=(et == 0), stop=(et == ET - 1),
                )
            mk = mk_p.tile([P, 512], dtb, tag="mk")
            nc.sync.dma_start(mk[:], maskt[jt * P:(jt + 1) * P,
                                           ib * 512: ib * 512 + 512])
            nc.vector.tensor_mul(st_blk[:, jt * 512:(jt + 1) * 512], ps[:], mk[:])
    for ib in range(I5):
        st_blk = st_blks[ib]
        for itl in range(4):
            g = ib * 4 + itl  # global i-tile
            for d5 in range(2):
                ps = ps_p.tile([P, 512], dtf, tag="psmm")
                for jt in range(JT):
                    nc.tensor.matmul(
                        ps[:],
                        lhsT=st_blk[:, jt * 512 + itl * P: jt * 512 + (itl + 1) * P],
                        rhs=vp_s[:, jt * D + d5 * 512: jt * D + d5 * 512 + 512],
                        start=(jt == 0), stop=(jt == JT - 1),
                    )
                ot = ev_p.tile([P, 512], dtf, tag="ev")
                nc.vector.tensor_scalar_mul(ot[:], ps[:], qinv_col[:, g:g + 1])
                nc.sync.dma_start(out[g * P:(g + 1) * P, d5 * 512: d5 * 512 + 512],
                                  ot[:])


def _build():
    if "nc" in _CACHE:
        return _CACHE["nc"]
    import concourse.tile as tile
    from concourse import bacc, mybir

    dtb = mybir.dt.bfloat16
    dtf = mybir.dt.float32
    nc = bacc.Bacc("TRN2", target_bir_lowering=False, debug=False,
                   enable_asserts=True, num_devices=N_CORES)
    xq = nc.dram_tensor("xq", [D, HALF], dtb, kind="ExternalInput").ap()
    wkt = nc.dram_tensor("wkt", [D, D], dtb, kind="ExternalInput").ap()
    wvt = nc.dram_tensor("wvt", [D, D], dtb, kind="ExternalInput").ap()
    wqt = nc.dram_tensor("wqt", [D, D], dtb, kind="ExternalInput").ap()
    maskt = nc.dram_tensor("maskt", [S, HALF], dtb, kind="ExternalInput").ap()
    out = nc.dram_tensor("out", [HALF, D], dtf, kind="ExternalOutput").ap()
    kt_own = nc.dram_tensor("kt_own", [D, HALF], dtb).ap()
    kt_gath = nc.dram_tensor("kt_gath", [2, D, HALF], dtb).ap()
    v_own = nc.dram_tensor("v_own", [HALF, D], dtb).ap()
    v_gath = nc.dram_tensor("v_gath", [2, HALF, D], dtb).ap()

    from contextlib import ExitStack
    with tile.TileContext(nc) as tc:
        with ExitStack() as ctx:
            _emit(ctx, tc, xq, wkt, wvt, wqt, maskt, out,
                  kt_own, kt_gath, v_own, v_gath)
    nc.compile()
    _CACHE["nc"] = nc
    return nc


def make_in_maps(X, masks, Wq, Wk, Wv):
    """Host-side sharding/layout: one input map per core (global key order)."""
    in_maps = []
    wkt_h = np.ascontiguousarray(Wk.T).astype(BF16)
    wvt_h = np.ascontiguousarray(Wv.T).astype(BF16)
    wqt_h = np.ascontiguousarray(Wq.T).astype(BF16)
    for c in range(N_CORES):
        b, h = c // 2, c % 2
        XT = X[b].T.astype(BF16)                                # [D, S]
        j = np.arange(S)[:, None]
        i = h * HALF + np.arange(HALF)[None, :]
        mT = ((j > i) | (masks[b] == 0)[:, None]).astype(BF16)  # [S, HALF]
        in_maps.append({
            "xq": np.ascontiguousarray(XT[:, h * HALF:(h + 1) * HALF]),
            "wkt": wkt_h,
            "wvt": wvt_h,
            "wqt": wqt_h,
            "maskt": mT,
        })
    return in_maps


def run(in_maps, **kw):
    from concourse.bass_utils import run_bass_kernel_spmd
    nc = _build()
    return run_bass_kernel_spmd(nc, in_maps, list(range(N_CORES)), **kw)


def kernel(X, masks, Wq, Wk, Wv):
    X = np.asarray(X, dtype=np.float32)
    masks = np.asarray(masks)
    res = run(make_in_maps(X, masks, np.asarray(Wq, np.float32),
                           np.asarray(Wk, np.float32), np.asarray(Wv, np.float32)))
    out = np.empty((B, S, D), np.float32)
    for c in range(N_CORES):
        b, h = c // 2, c % 2
        out[b, h * HALF:(h + 1) * HALF, :] = res.results[c]["out"]
    return out


# revision 16
# speedup vs baseline: 1.0716x; 1.0138x over previous
"""Bass/Trainium2 kernel for nn_Attention_75007308857927.

Reference computation (B=4, S=2048, D=1024):
    Q = X @ Wq.T ; K = X @ Wk.T ; V = X @ Wv.T         (per batch)
    Qn, Kn = row-normalized Q, K
    scores = (Qn @ Kn.T) * m      m[i,j] = 1 if (j > i) or masks[j]==0 else 0
    out = scores @ V

Sharding: 8 cores = 4 batches x 2 query/key-halves. Each core projects
K/V/Q from its own 1024-row half of X; the full KT and V' (V scaled by
1/||K||) are assembled across the core pair with an AllGather, then each
core computes scores/out for its query half.

Device algebra per core (matmuls contract over the partition dim):
    KT[e,j']  = sum_d WkT[d,e] * XQ[d,j']        (own keys j', KT e-major)
    kinv[j']  = rsqrt(sum_e KT[e,j']^2)
    V'[j',e]  = (sum_d XQ[d,j'] WvT[d,e]) * kinv[j']
    KT, V'    = AllGather over the core pair     (global key order)
    QT[e,i]   = sum_d WqT[d,e] * XQ[d,i]
    qinv[i]   = rsqrt(sum_e QT[e,i]^2)
    ST[j,i]   = (sum_e KT[e,j] QT[e,i]) * maskT[j,i]   (mask fused in evict)
    out[i,d]  = (sum_j ST[j,i] V'[j,d]) * qinv[i]

bf16 matmul operands, f32 PSUM accumulation. Precision vs f32 reference:
absmax error ~0.4% of output scale.
"""

import numpy as np
import ml_dtypes

B, S, D = 4, 2048, 1024
HALF = S // 2  # queries/keys per core
N_CORES = 8
P = 128
DC = D // P    # 8 contraction chunks over d
ET = D // P    # 8 e-tiles
JT = S // P    # 16 j-tiles (global)
JTH = HALF // P  # 8 own j-tiles
I5 = HALF // 512  # 2

BF16 = ml_dtypes.bfloat16

_CACHE = {}


def _emit(ctx, tc, xq, wkt, wvt, wqt, maskt, out, kt_own, kt_gath, v_own, v_gath):
    from concourse import mybir

    nc = tc.nc
    dtb = mybir.dt.bfloat16
    dtf = mybir.dt.float32

    # ---- SBUF pools -------------------------------------------------------
    xq_p = ctx.enter_context(tc.tile_pool(name="xq", bufs=1))
    # weights + scores blocks share one pool: all tiles are 16KB/partition
    w_p = ctx.enter_context(tc.tile_pool(name="wst", bufs=3))
    kt_p = ctx.enter_context(tc.tile_pool(name="kt", bufs=1))
    qt_p = ctx.enter_context(tc.tile_pool(name="qt", bufs=1))
    vp_p = ctx.enter_context(tc.tile_pool(name="vp", bufs=1))
    row_p = ctx.enter_context(tc.tile_pool(name="rows", bufs=1))
    sq_p = ctx.enter_context(tc.tile_pool(name="sq", bufs=3))
    stg_p = ctx.enter_context(tc.tile_pool(name="stg", bufs=6))
    mk_p = ctx.enter_context(tc.tile_pool(name="mk", bufs=8))
    ev_p = ctx.enter_context(tc.tile_pool(name="ev", bufs=3))
    ps_p = ctx.enter_context(tc.tile_pool(name="psmm", bufs=5, space="PSUM"))
    psr_p = ctx.enter_context(tc.tile_pool(name="psrow", bufs=2, space="PSUM"))
    psc_p = ctx.enter_context(tc.tile_pool(name="pscol", bufs=1, space="PSUM"))

    xq_s = xq_p.tile([P, DC * HALF], dtb, tag="xq")    # [d%128, dc*1024+i]
    wkt_s = w_p.tile([P, DC * D], dtb, tag="w")        # [d%128, dc*1024+e]
    wvt_s = w_p.tile([P, DC * D], dtb, tag="w")
    wqt_s = w_p.tile([P, DC * D], dtb, tag="w")
    kt_s = kt_p.tile([P, ET * S], dtb, tag="kt")       # [e%128, et*2048+j]
    qt_s = qt_p.tile([P, ET * HALF], dtb, tag="qt")    # [e%128, et*1024+i]
    vp_s = vp_p.tile([P, JT * D], dtb, tag="vp")       # [j%128, jt*1024+d]

    ones_b = row_p.tile([P, 1], dtb, tag="ones_b")
    ones_f = row_p.tile([1, 1], dtf, tag="ones_f")
    ksq_row = row_p.tile([1, HALF], dtf, tag="sqrow")
    qsq_row = row_p.tile([1, HALF], dtf, tag="sqrow")
    ksq_col = row_p.tile([P, JTH], dtf, tag="ksqc")    # col c <-> own j-chunk c
    krec_col = row_p.tile([P, JTH], dtf, tag="krecc")
    kinv_col = row_p.tile([P, JTH], dtf, tag="kinvc")
    qsq_col = row_p.tile([P, ET], dtf, tag="qsqc")
    qrec_col = row_p.tile([P, ET], dtf, tag="qrecc")
    qinv_col = row_p.tile([P, ET], dtf, tag="qinvc")

    nc.vector.memset(ones_b[:], 1.0)
    nc.vector.memset(ones_f[:], 1.0)

    # ---- input DMAs (ordered for earliest matmul start: B needs wkt+xq) --
    for dc in range(DC):
        nc.sync.dma_start(wkt_s[:, dc * D:(dc + 1) * D], wkt[dc * P:(dc + 1) * P, :])
        nc.scalar.dma_start(xq_s[:, dc * HALF:(dc + 1) * HALF],
                            xq[dc * P:(dc + 1) * P, :])
    for w_s, w_d in ((wvt_s, wvt), (wqt_s, wqt)):
        for dc in range(DC):
            nc.sync.dma_start(w_s[:, dc * D:(dc + 1) * D], w_d[dc * P:(dc + 1) * P, :])

    groups = [[0, 1], [2, 3], [4, 5], [6, 7]]

    # ---- phase B: KT for own keys + k sumsq ------------------------------
    for j5 in range(I5):
        ksq_ps = psr_p.tile([1, 512], dtf, tag="psrow")
        for et in range(ET):
            ps = ps_p.tile([P, 512], dtf, tag="psmm")
            for dc in range(DC):
                nc.tensor.matmul(
                    ps[:],
                    lhsT=wkt_s[:, dc * D + et * P: dc * D + (et + 1) * P],
                    rhs=xq_s[:, dc * HALF + j5 * 512: dc * HALF + j5 * 512 + 512],
                    start=(dc == 0), stop=(dc == DC - 1),
                )
            stg = stg_p.tile([P, 512], dtb, tag="stg")
            nc.vector.tensor_copy(stg[:], ps[:])
            nc.sync.dma_start(kt_own[et * P:(et + 1) * P, j5 * 512: j5 * 512 + 512],
                              stg[:])
            sq = sq_p.tile([P, 512], dtb, tag="sq")
            nc.scalar.square(sq[:], ps[:])
            nc.tensor.matmul(ksq_ps[:], lhsT=ones_b[:], rhs=sq[:],
                             start=(et == 0), stop=(et == ET - 1))
        nc.vector.tensor_copy(ksq_row[0:1, j5 * 512: j5 * 512 + 512], ksq_ps[:])
        for cc in range(4):
            c = j5 * 4 + cc
            pc = psc_p.tile([P, 1], dtf, tag="pscol")
            nc.tensor.matmul(pc[:], lhsT=ksq_row[0:1, c * P:(c + 1) * P],
                             rhs=ones_f[:], start=True, stop=True)
            nc.vector.tensor_copy(ksq_col[:, c:c + 1], pc[:])

    # kinv for own keys, 128-way parallel in column layout
    nc.vector.reciprocal(krec_col[:], ksq_col[:])
    nc.scalar.sqrt(kinv_col[:], krec_col[:])

    # gather KT across the core pair, then load full KT to SBUF
    nc.gpsimd.collective_compute(
        "AllGather", mybir.AluOpType.bypass, replica_groups=groups,
        ins=[kt_own[:]], outs=[kt_gath[:]])
    kt3 = kt_s[:].rearrange("p (et j) -> p et j", et=ET, j=S)
    for r in range(2):
        src3 = kt_gath[r].rearrange("(et p) j -> p et j", p=P)
        for eg in range(0, ET, 2):
            nc.gpsimd.dma_start(
                kt3[:, eg:eg + 2, r * HALF:(r + 1) * HALF],
                src3[:, eg:eg + 2, :])

    # ---- phase D: V' = V * kinv[j] for own keys --------------------------
    for jt in range(JTH):
        for e5 in range(2):
            ps = ps_p.tile([P, 512], dtf, tag="psmm")
            for dc in range(DC):
                nc.tensor.matmul(
                    ps[:],
                    lhsT=xq_s[:, dc * HALF + jt * P: dc * HALF + (jt + 1) * P],
                    rhs=wvt_s[:, dc * D + e5 * 512: dc * D + e5 * 512 + 512],
                    start=(dc == 0), stop=(dc == DC - 1),
                )
            stg = stg_p.tile([P, 512], dtb, tag="stg")
            nc.vector.tensor_scalar_mul(stg[:], ps[:], kinv_col[:, jt:jt + 1])
            nc.sync.dma_start(v_own[jt * P:(jt + 1) * P, e5 * 512: e5 * 512 + 512],
                              stg[:])

    nc.gpsimd.collective_compute(
        "AllGather", mybir.AluOpType.bypass, replica_groups=groups,
        ins=[v_own[:]], outs=[v_gath[:]])
    for r in range(2):
        dst = vp_s[:, r * JTH * D: (r * JTH + JTH) * D]
        dst = dst.rearrange("p (jtl e) -> p jtl e", jtl=JTH, e=D)
        src_ap = v_gath[r].rearrange("(jtl p) e -> p jtl e", p=P)
        for jg in range(0, JTH, 2):
            nc.gpsimd.dma_start(dst[:, jg:jg + 2, :], src_ap[:, jg:jg + 2, :])

    # ---- phase E: QT + q sumsq -------------------------------------------
    for i5 in range(I5):
        qsq_ps = psr_p.tile([1, 512], dtf, tag="psrow")
        for et in range(ET):
            ps = ps_p.tile([P, 512], dtf, tag="psmm")
            for dc in range(DC):
                nc.tensor.matmul(
                    ps[:],
                    lhsT=wqt_s[:, dc * D + et * P: dc * D + (et + 1) * P],
                    rhs=xq_s[:, dc * HALF + i5 * 512: dc * HALF + i5 * 512 + 512],
                    start=(dc == 0), stop=(dc == DC - 1),
                )
            qtsl = qt_s[:, et * HALF + i5 * 512: et * HALF + i5 * 512 + 512]
            nc.vector.tensor_copy(qtsl, ps[:])
            sq = sq_p.tile([P, 512], dtb, tag="sq")
            nc.scalar.square(sq[:], ps[:])
            nc.tensor.matmul(qsq_ps[:], lhsT=ones_b[:], rhs=sq[:],
                             start=(et == 0), stop=(et == ET - 1))
        nc.vector.tensor_copy(qsq_row[0:1, i5 * 512: i5 * 512 + 512], qsq_ps[:])

    # ---- phase F: all score blocks first (hides the V' gather), then ------
    # ---- phase G: all out blocks ------------------------------------------
    st_blks = []
    for ib in range(I5):
        st_blk = w_p.tile([P, JT * 512], dtb, tag="w")  # [j%128, jt*512+i]
        st_blks.append(st_blk)
        for jt in range(JT):
            ps = ps_p.tile([P, 512], dtf, tag="psmm")
            for et in range(ET):
                nc.tensor.matmul(
                    ps[:],
                    lhsT=kt_s[:, et * S + jt * P: et * S + (jt + 1) * P],
                    rhs=qt_s[:, et * HALF + ib * 512: et * HALF + ib * 512 + 512],
                    start=(et == 0), stop=(et == ET - 1),
                )
            mk = mk_p.tile([P, 512], dtb, tag="mk")
            nc.sync.dma_start(mk[:], maskt[jt * P:(jt + 1) * P,
                                           ib * 512: ib * 512 + 512])
            nc.vector.tensor_mul(st_blk[:, jt * 512:(jt + 1) * 512], ps[:], mk[:])
    # q-norm chain (deferred so F's matmuls aren't blocked behind it)
    for c in range(ET):
        pc = psc_p.tile([P, 1], dtf, tag="pscol")
        nc.tensor.matmul(pc[:], lhsT=qsq_row[0:1, c * P:(c + 1) * P],
                         rhs=ones_f[:], start=True, stop=True)
        nc.vector.tensor_copy(qsq_col[:, c:c + 1], pc[:])
    nc.vector.reciprocal(qrec_col[:], qsq_col[:])
    nc.scalar.sqrt(qinv_col[:], qrec_col[:])

    for ib in range(I5):
        st_blk = st_blks[ib]
        for itl in range(4):
            g = ib * 4 + itl  # global i-tile
            for d5 in range(2):
                ps = ps_p.tile([P, 512], dtf, tag="psmm")
                for jt in range(JT):
                    nc.tensor.matmul(
                        ps[:],
                        lhsT=st_blk[:, jt * 512 + itl * P: jt * 512 + (itl + 1) * P],
                        rhs=vp_s[:, jt * D + d5 * 512: jt * D + d5 * 512 + 512],
                        start=(jt == 0), stop=(jt == JT - 1),
                    )
                ot = ev_p.tile([P, 512], dtf, tag="ev")
                nc.vector.tensor_scalar_mul(ot[:], ps[:], qinv_col[:, g:g + 1])
                nc.sync.dma_start(out[g * P:(g + 1) * P, d5 * 512: d5 * 512 + 512],
                                  ot[:])


def _build():
    if "nc" in _CACHE:
        return _CACHE["nc"]
    import concourse.tile as tile
    from concourse import bacc, mybir

    dtb = mybir.dt.bfloat16
    dtf = mybir.dt.float32
    nc = bacc.Bacc("TRN2", target_bir_lowering=False, debug=False,
                   enable_asserts=True, num_devices=N_CORES)
    xq = nc.dram_tensor("xq", [D, HALF], dtb, kind="ExternalInput").ap()
    wkt = nc.dram_tensor("wkt", [D, D], dtb, kind="ExternalInput").ap()
    wvt = nc.dram_tensor("wvt", [D, D], dtb, kind="ExternalInput").ap()
    wqt = nc.dram_tensor("wqt", [D, D], dtb, kind="ExternalInput").ap()
    maskt = nc.dram_tensor("maskt", [S, HALF], dtb, kind="ExternalInput").ap()
    out = nc.dram_tensor("out", [HALF, D], dtf, kind="ExternalOutput").ap()
    kt_own = nc.dram_tensor("kt_own", [D, HALF], dtb).ap()
    kt_gath = nc.dram_tensor("kt_gath", [2, D, HALF], dtb).ap()
    v_own = nc.dram_tensor("v_own", [HALF, D], dtb).ap()
    v_gath = nc.dram_tensor("v_gath", [2, HALF, D], dtb).ap()

    from contextlib import ExitStack
    with tile.TileContext(nc) as tc:
        with ExitStack() as ctx:
            _emit(ctx, tc, xq, wkt, wvt, wqt, maskt, out,
                  kt_own, kt_gath, v_own, v_gath)
    nc.compile()
    _CACHE["nc"] = nc
    return nc


def make_in_maps(X, masks, Wq, Wk, Wv):
    """Host-side sharding/layout: one input map per core (global key order)."""
    in_maps = []
    wkt_h = np.ascontiguousarray(Wk.T).astype(BF16)
    wvt_h = np.ascontiguousarray(Wv.T).astype(BF16)
    wqt_h = np.ascontiguousarray(Wq.T).astype(BF16)
    for c in range(N_CORES):
        b, h = c // 2, c % 2
        XT = X[b].T.astype(BF16)                                # [D, S]
        j = np.arange(S)[:, None]
        i = h * HALF + np.arange(HALF)[None, :]
        mT = ((j > i) | (masks[b] == 0)[:, None]).astype(BF16)  # [S, HALF]
        in_maps.append({
            "xq": np.ascontiguousarray(XT[:, h * HALF:(h + 1) * HALF]),
            "wkt": wkt_h,
            "wvt": wvt_h,
            "wqt": wqt_h,
            "maskt": mT,
        })
    return in_maps


def run(in_maps, **kw):
    from concourse.bass_utils import run_bass_kernel_spmd
    nc = _build()
    return run_bass_kernel_spmd(nc, in_maps, list(range(N_CORES)), **kw)


def kernel(X, masks, Wq, Wk, Wv):
    X = np.asarray(X, dtype=np.float32)
    masks = np.asarray(masks)
    res = run(make_in_maps(X, masks, np.asarray(Wq, np.float32),
                           np.asarray(Wk, np.float32), np.asarray(Wv, np.float32)))
    out = np.empty((B, S, D), np.float32)
    for c in range(N_CORES):
        b, h = c // 2, c % 2
        out[b, h * HALF:(h + 1) * HALF, :] = res.results[c]["out"]
    return out
